# revision 12
# baseline (speedup 1.0000x reference)
"""MultiHeadAttention Trainium2 kernel (8 NeuronCores, SPMD), v2.

Problem: B=2, L=2048, DK=DV=512, H=8, dh=64.
  Q = q @ WQ[h]; K = k @ WK[h]; V = v @ WV[h]       (per head)
  y = Q K^T / sqrt(L); z = softmax(y, axis=QUERY); out = z @ V
  concat heads on feature dim.

Sharding: 16 (b,h) pairs over 8 cores -> 2 heads (same batch) per core.

v2 changes vs v1 (81.3us):
  * Scores matmuls run in fp8e4m3 with perf_mode=DoubleRow at 0.5
    cycles/row (half the bf16 PE time).  The 64-deep head contraction is
    packed to the required 2-per-partition pair layout with stride-0
    broadcast APs (each feature streamed twice -> computes 2*y; the 1/2
    is folded into the exp scale).  Q/K projections evacuate straight to
    fp8 (no bf16 Q/K ever materialized).
  * Scores PSUM tiles are [128, 1024] (2 banks); exp runs as halves,
    amortizing the fixed ACT/DVE access-latency + accum-read costs.
  * exp blocks (kt, h) are split ACT-exact vs DVE-Schraudolph via
    ACT_BLOCKS; DVE block D-sums run in-place (out == in) at 4x mode.
  * V projection batched 4 k-tiles per PSUM tile, one [128,512] ACT
    evac instead of 4 x [128,128].
  * Vs = Vf * (1/D) scaling moved to the (otherwise idle) Pool engine.
  * PSUM: 2 x [128,1024] score tiles (also hosting Q/K/V projections)
    + 4-bank AV accumulator = exactly 8 banks.
"""

import math

import numpy as np

B = 2
L = 2048
DK = 512
H = 8
DH = 64
P = 128
NKT = L // P  # 16 k-tiles
NDC = DK // P  # 4 d-chunks
N_CORES = 8

SCALE = 1.0 / math.sqrt(float(L))
# Schraudolph in bf16-bits domain: round(raw*EXP_A2 + EXP_B) as int16,
# bitcast bf16 ~= exp(raw*SCALE).  Raw fp8 scores are 2*y so EXP_A2
# carries a 1/2.
EXP_A2 = 128.0 * math.log2(math.e) * SCALE * 0.5
EXP_B = 16256.0 - 12.0
SCALE2 = SCALE * 0.5  # ACT exp scale on the doubled raw scores

# (kt, h) blocks whose exp runs exact on ACT with fused accum D; the
# rest run DVE Schraudolph (~2% rel, bias common-mode cancels in E/D).
ACT_BLOCKS = frozenset((kt, 0) for kt in range(NKT)) | {(5, 1)}

E_BUFS = 6
VF_BUFS = 2
VS_BUFS = 4

_CACHE = {}


def _build_program():
    import concourse.bass as bass
    import concourse.tile as tile
    from concourse import bacc, mybir
    from concourse.bass import ts

    f32 = mybir.dt.float32
    bf16 = mybir.dt.bfloat16
    fp8 = mybir.dt.float8e4
    i16 = mybir.dt.int16
    AF = mybir.ActivationFunctionType
    ALU = mybir.AluOpType
    DR = mybir.MatmulPerfMode.DoubleRow

    nc = bacc.Bacc("TRN2", target_bir_lowering=False, debug=False)

    qt_d = nc.dram_tensor("qt", [DK, L], bf16, kind="ExternalInput")
    kt_d = nc.dram_tensor("kt", [DK, L], bf16, kind="ExternalInput")
    vt_d = nc.dram_tensor("vt", [DK, L], bf16, kind="ExternalInput")
    wq_d = nc.dram_tensor("wq", [DK, P], bf16, kind="ExternalInput")
    wk_d = nc.dram_tensor("wk", [DK, P], bf16, kind="ExternalInput")
    wv_d = nc.dram_tensor("wv", [DK, P], bf16, kind="ExternalInput")
    out_d = nc.dram_tensor("out", [P, NKT, P], bf16, kind="ExternalOutput")

    with tile.TileContext(nc) as tc:
        with (
            tc.tile_pool(name="consts", bufs=1) as consts,
            tc.tile_pool(name="xin", bufs=1) as xin,
            tc.tile_pool(name="proj", bufs=1) as proj,
            tc.tile_pool(name="epool", bufs=E_BUFS) as epool,
            tc.tile_pool(name="vfpool", bufs=VF_BUFS) as vfpool,
            tc.tile_pool(name="scrpool", bufs=2) as scrpool,
            tc.tile_pool(name="vspool", bufs=VS_BUFS) as vspool,
            tc.tile_pool(name="stats", bufs=1) as stats,
            tc.tile_pool(name="outp", bufs=1) as outp,
            tc.tile_pool(name="spsum", bufs=2, space="PSUM") as spsum,
            tc.tile_pool(name="avpsum", bufs=1, space="PSUM") as avpsum,
        ):
            wq_s = consts.tile([P, NDC, P], bf16)
            wk_s = consts.tile([P, NDC, P], bf16)
            wv_s = consts.tile([P, NDC, P], bf16)
            qt_s = xin.tile([P, NDC, L], bf16)
            kt_s = xin.tile([P, NDC, L], bf16)
            vt_s = xin.tile([P, NDC, L], bf16)
            qt_r = qt_d.rearrange("(o p) l -> p o l", p=P)
            kt_r = kt_d.rearrange("(o p) l -> p o l", p=P)
            vt_r = vt_d.rearrange("(o p) l -> p o l", p=P)

            def load_chunk(sb, rr, c):
                nc.sync.dma_start(sb[:, :, ts(c, 512)], rr[:, :, ts(c, 512)])

            # PE p-state: tiny dummy matmul at t~0 starts the 3us ramp early
            zt = consts.tile([P, P], bf16)
            nc.gpsimd.memset(zt[:], 0.0)
            pwp = spsum.tile([P, 1024], f32, tag="sco", name="prewarm")
            nc.tensor.matmul(pwp[:, 0:P], lhsT=zt[:], rhs=zt[:], start=True, stop=True)

            # critical-path-first load order
            nc.sync.dma_start(wq_s[:], wq_d.rearrange("(o p) e -> p o e", p=P))
            load_chunk(qt_s, qt_r, 0)
            nc.sync.dma_start(wk_s[:], wk_d.rearrange("(o p) e -> p o e", p=P))
            load_chunk(kt_s, kt_r, 0)
            load_chunk(qt_s, qt_r, 1)
            load_chunk(qt_s, qt_r, 2)
            load_chunk(qt_s, qt_r, 3)
            nc.sync.dma_start(wv_s[:], wv_d.rearrange("(o p) e -> p o e", p=P))
            load_chunk(vt_s, vt_r, 0)
            load_chunk(kt_s, kt_r, 1)
            load_chunk(kt_s, kt_r, 2)
            load_chunk(vt_s, vt_r, 1)
            load_chunk(kt_s, kt_r, 3)
            load_chunk(vt_s, vt_r, 2)
            load_chunk(vt_s, vt_r, 3)

            # fp8 projected Q/K: head h features on partitions h*64..h*64+64
            QT8 = proj.tile([P, L], fp8)
            KT8 = proj.tile([P, L], fp8)

            # per-(kt,h) denominator sums; [P, kt*2 + h] pairs adjacent for
            # the reciprocal
            Dsum2 = stats.tile([P, NKT, 2, 2], f32)  # ACT accum halves [kt,h,hf]
            Dtot = stats.tile([P, NKT * 2], f32)
            Drec = stats.tile([P, NKT * 2], f32)

            # AV accumulator: out[q(128), qt(16), ev-pack(128)] f32 = 4 banks
            avpA = avpsum.tile([P, 8, P], f32, tag="ava")
            avpB = avpsum.tile([P, 8, P], f32, tag="avb")

            def qk_proj_half(W, X, OUT8, hf, on_act=True):
                # project q-cols [hf*1024, hf*1024+1024): 2 chunks of 512
                ps = spsum.tile([P, 1024], f32, tag="sco", name="qkproj")
                for sub in range(2):
                    qc = hf * 2 + sub
                    for dc in range(NDC):
                        nc.tensor.matmul(
                            ps[:, ts(sub, 512)],
                            lhsT=W[:, dc, :],
                            rhs=X[:, dc, ts(qc, 512)],
                            start=(dc == 0),
                            stop=(dc == NDC - 1),
                        )
                if on_act:
                    nc.scalar.copy(OUT8[:, ts(hf, 1024)], ps[:])
                else:
                    nc.vector.tensor_copy(OUT8[:, ts(hf, 1024)], ps[:])

            Etiles = {}
            Vstiles = {}

            def scores_half(kt, h, hf, E):
                # one [128, 1024] half: 2 DoubleRow fp8 matmuls + one exp
                hp = h * DH
                ps = spsum.tile([P, 1024], f32, tag="sco", name="sco")
                for sub in range(2):
                    qc = hf * 2 + sub
                    lhs = (
                        KT8[hp : hp + DH, ts(kt, P)]
                        .unsqueeze(1)
                        .broadcast_to([DH, 2, P])
                    )
                    rhs = (
                        QT8[hp : hp + DH, ts(qc, 512)]
                        .unsqueeze(1)
                        .broadcast_to([DH, 2, 512])
                    )
                    nc.tensor.matmul(
                        ps[:, ts(sub, 512)],
                        lhsT=lhs,
                        rhs=rhs,
                        start=True,
                        stop=True,
                        perf_mode=DR,
                    )
                if (kt, h) in ACT_BLOCKS:
                    nc.scalar.activation(
                        E[:, ts(hf, 1024)],
                        ps[:],
                        AF.Exp,
                        scale=SCALE2,
                        accum_out=Dsum2[:, kt : kt + 1, h : h + 1, hf : hf + 1],
                    )
                else:
                    e16 = E[:, ts(hf, 1024)].bitcast(i16)
                    nc.vector.tensor_scalar(
                        e16, ps[:], EXP_A2, EXP_B, ALU.mult, ALU.add
                    )

            def evac_half_a():
                oa = outp.tile([P, 8, P], bf16, tag="oca")
                nc.scalar.copy(oa[:], avpA[:])
                nc.scalar.dma_start(out_d[:, 0:8, :], oa[:])

            def evac_half_b():
                ob = outp.tile([P, 8, P], bf16, tag="ocb")
                nc.vector.tensor_copy(ob[:], avpB[:])
                nc.sync.dma_start(out_d[:, 8:16, :], ob[:])

            def av_chunk(kt, qlo, qhi, tail=False):
                E0, E1 = Etiles[kt]
                Vs = Vstiles[kt]
                for qt in range(qlo, qhi):
                    av = avpA if qt < 8 else avpB
                    qi = qt % 8
                    for h in range(2):
                        E = E0 if h == 0 else E1
                        # start=True zeroes the whole PSUM bank (4 q-tiles),
                        # so only the first matmul per bank may carry it
                        nc.tensor.matmul(
                            av[:, qi, ts(h, DH)],
                            lhsT=E[:, ts(qt, P)],
                            rhs=Vs[:, ts(h, DH)],
                            start=(kt == 0 and h == 0 and qt % 4 == 0),
                            stop=(kt == NKT - 1),
                            skip_group_check=True,
                        )
                    if tail and qt == 7:
                        evac_half_a()

            def vproj_group(g):
                # V projection for k-tiles 4g..4g+3 in one [128,512] psum
                psv = spsum.tile([P, 1024], f32, tag="sco", name="psv")
                for j in range(4):
                    # j=0 / j=2 start=True zeroes banks 0 / 1 of the tile;
                    # j=1 / j=3 accumulate onto the already-zeroed bank
                    for dc in range(NDC):
                        nc.tensor.matmul(
                            psv[:, ts(j, P)],
                            lhsT=vt_s[:, dc, ts(4 * g + j, P)],
                            rhs=wv_s[:, dc, :],
                            start=(j % 2 == 0 and dc == 0),
                            stop=(dc == NDC - 1),
                            skip_group_check=True,
                        )
                Vf = vfpool.tile([P, 512], bf16, tag="vf")
                nc.scalar.copy(Vf[:], psv[:, 0:512])
                return Vf

            def d_vs_block(kt, Vf, voff):
                # D: ACT blocks from fused accum halves (Pool adds them);
                # DVE blocks from an in-place 4x bf16 accum pass over E
                for h in range(2):
                    E = Etiles[kt][h]
                    if (kt, h) in ACT_BLOCKS:
                        nc.gpsimd.tensor_add(
                            Dtot[:, 2 * kt + h : 2 * kt + h + 1],
                            Dsum2[:, kt, h, 0:1],
                            Dsum2[:, kt, h, 1:2],
                        )
                    else:
                        # separate scr output (not in-place): keeps AV's E
                        # reads off the D-pass dependency
                        scr = scrpool.tile([P, L], bf16, tag="scr")
                        nc.vector.tensor_scalar(
                            scr[:],
                            E[:],
                            1.0,
                            0.0,
                            ALU.mult,
                            ALU.add,
                            accum_out=Dtot[:, 2 * kt + h : 2 * kt + h + 1],
                        )
                nc.vector.reciprocal(
                    Drec[:, 2 * kt : 2 * kt + 2], Dtot[:, 2 * kt : 2 * kt + 2]
                )
                Vs = vspool.tile([P, P], bf16, tag="vs")
                Vstiles[kt] = Vs
                for h in range(2):
                    # Pool: Vs = Vf * (1/D), per-partition ptr scalar
                    nc.gpsimd.tensor_scalar_mul(
                        Vs[:, ts(h, DH)],
                        Vf[:, voff + h * DH : voff + (h + 1) * DH],
                        Drec[:, 2 * kt + h : 2 * kt + h + 1],
                    )

            def alloc_E(kt):
                E0 = epool.tile([P, L], bf16, tag="E", name=f"E{kt}h0")
                E1 = epool.tile([P, L], bf16, tag="E", name=f"E{kt}h1")
                Etiles[kt] = (E0, E1)

            # warmup: Q halves then K halves rotate through the 2 spsum bufs
            qk_proj_half(wq_s, qt_s, QT8, 0, on_act=True)
            qk_proj_half(wq_s, qt_s, QT8, 1, on_act=True)
            qk_proj_half(wk_s, kt_s, KT8, 0, on_act=True)

            Vfgroups = {}

            for kt in range(NKT):
                alloc_E(kt)
                E0, E1 = Etiles[kt]
                if kt == 0:
                    scores_half(0, 0, 0, E0)
                    qk_proj_half(wk_s, kt_s, KT8, 1, on_act=False)
                    scores_half(0, 0, 1, E0)
                    scores_half(0, 1, 0, E1)
                    scores_half(0, 1, 1, E1)
                    Vfgroups[0] = vproj_group(0)
                    d_vs_block(0, Vfgroups[0], 0)
                else:
                    # interleave: AV chunks of kt-1 fill the PE queue while
                    # the exp engines drain the first-half psum tiles, so the
                    # in-order PE never parks ahead of ready AV work
                    scores_half(kt, 0, 0, E0)
                    scores_half(kt, 1, 0, E1)
                    av_chunk(kt - 1, 0, 8)
                    scores_half(kt, 0, 1, E0)
                    scores_half(kt, 1, 1, E1)
                    g, r = divmod(kt + 1, 4)
                    if r == 0 and g < 4:
                        Vfgroups[g] = vproj_group(g)
                    av_chunk(kt - 1, 8, 16)
                    d_vs_block(kt, Vfgroups[kt // 4], (kt % 4) * P)

            av_chunk(NKT - 1, 0, 8, tail=True)
            av_chunk(NKT - 1, 8, 16, tail=True)
            evac_half_b()

    nc.compile()
    return nc


def _get_program():
    if "nc" not in _CACHE:
        _CACHE["nc"] = _build_program()
    return _CACHE["nc"]


def kernel(keys, queries, values, WQ, WK, WV):
    import ml_dtypes

    from concourse import bass_utils

    bf = ml_dtypes.bfloat16
    keys = np.asarray(keys)
    queries = np.asarray(queries)
    values = np.asarray(values)
    WQ = np.asarray(WQ)
    WK = np.asarray(WK)
    WV = np.asarray(WV)

    nc = _get_program()

    in_maps = []
    for c in range(N_CORES):
        b = c // 4
        h0 = 2 * (c % 4)
        h1 = h0 + 1
        in_maps.append(
            {
                "qt": np.ascontiguousarray(queries[b].T).astype(bf),
                "kt": np.ascontiguousarray(keys[b].T).astype(bf),
                "vt": np.ascontiguousarray(values[b].T).astype(bf),
                "wq": np.concatenate([WQ[h0], WQ[h1]], axis=1).astype(bf),
                "wk": np.concatenate([WK[h0], WK[h1]], axis=1).astype(bf),
                "wv": np.concatenate([WV[h0], WV[h1]], axis=1).astype(bf),
            }
        )

    res = bass_utils.run_bass_kernel_spmd(nc, in_maps, core_ids=list(range(N_CORES)))

    out = np.empty((B, L, H * DH), dtype=np.float32)
    for c in range(N_CORES):
        b = c // 4
        h0 = 2 * (c % 4)
        ot = np.asarray(res.results[c]["out"], dtype=np.float32)  # [128,16,128]
        out[b, :, h0 * DH : (h0 + 2) * DH] = ot.transpose(1, 0, 2).reshape(L, P)
    return out


# revision 13
# speedup vs baseline: 1.0337x; 1.0337x over previous
"""MultiHeadAttention Trainium2 kernel (8 NeuronCores, SPMD), v2.

Problem: B=2, L=2048, DK=DV=512, H=8, dh=64.
  Q = q @ WQ[h]; K = k @ WK[h]; V = v @ WV[h]       (per head)
  y = Q K^T / sqrt(L); z = softmax(y, axis=QUERY); out = z @ V
  concat heads on feature dim.

Sharding: 16 (b,h) pairs over 8 cores -> 2 heads (same batch) per core.

v2 changes vs v1 (81.3us):
  * Scores matmuls run in fp8e4m3 with perf_mode=DoubleRow at 0.5
    cycles/row (half the bf16 PE time).  The 64-deep head contraction is
    packed to the required 2-per-partition pair layout with stride-0
    broadcast APs (each feature streamed twice -> computes 2*y; the 1/2
    is folded into the exp scale).  Q/K projections evacuate straight to
    fp8 (no bf16 Q/K ever materialized).
  * Scores PSUM tiles are [128, 1024] (2 banks); exp runs as halves,
    amortizing the fixed ACT/DVE access-latency + accum-read costs.
  * exp blocks (kt, h) are split ACT-exact vs DVE-Schraudolph via
    ACT_BLOCKS; DVE block D-sums run in-place (out == in) at 4x mode.
  * V projection batched 4 k-tiles per PSUM tile, one [128,512] ACT
    evac instead of 4 x [128,128].
  * Vs = Vf * (1/D) scaling moved to the (otherwise idle) Pool engine.
  * PSUM: 2 x [128,1024] score tiles (also hosting Q/K/V projections)
    + 4-bank AV accumulator = exactly 8 banks.
"""

import math

import numpy as np

B = 2
L = 2048
DK = 512
H = 8
DH = 64
P = 128
NKT = L // P  # 16 k-tiles
NDC = DK // P  # 4 d-chunks
N_CORES = 8

SCALE = 1.0 / math.sqrt(float(L))
# Schraudolph in bf16-bits domain: round(raw*EXP_A2 + EXP_B) as int16,
# bitcast bf16 ~= exp(raw*SCALE).  Raw fp8 scores are 2*y so EXP_A2
# carries a 1/2.
EXP_A2 = 128.0 * math.log2(math.e) * SCALE * 0.5
EXP_B = 16256.0 - 12.0
SCALE2 = SCALE * 0.5  # ACT exp scale on the doubled raw scores

# (kt, h) blocks whose exp runs exact on ACT with fused accum D; the
# rest run DVE Schraudolph (~2% rel, bias common-mode cancels in E/D).
ACT_BLOCKS = frozenset((kt, 0) for kt in range(NKT)) | {(5, 1)}

E_BUFS = 6
VF_BUFS = 2
VS_BUFS = 4

_CACHE = {}


def _build_program():
    import concourse.bass as bass
    import concourse.tile as tile
    from concourse import bacc, mybir
    from concourse.bass import ts

    f32 = mybir.dt.float32
    bf16 = mybir.dt.bfloat16
    fp8 = mybir.dt.float8e4
    i16 = mybir.dt.int16
    AF = mybir.ActivationFunctionType
    ALU = mybir.AluOpType
    DR = mybir.MatmulPerfMode.DoubleRow

    nc = bacc.Bacc("TRN2", target_bir_lowering=False, debug=False)

    qt_d = nc.dram_tensor("qt", [DK, L], bf16, kind="ExternalInput")
    kt_d = nc.dram_tensor("kt", [DK, L], bf16, kind="ExternalInput")
    vt_d = nc.dram_tensor("vt", [DK, L], bf16, kind="ExternalInput")
    wq_d = nc.dram_tensor("wq", [DK, P], bf16, kind="ExternalInput")
    wk_d = nc.dram_tensor("wk", [DK, P], bf16, kind="ExternalInput")
    wv_d = nc.dram_tensor("wv", [DK, P], bf16, kind="ExternalInput")
    out_d = nc.dram_tensor("out", [P, NKT, P], bf16, kind="ExternalOutput")

    with tile.TileContext(nc) as tc:
        with (
            tc.tile_pool(name="consts", bufs=1) as consts,
            tc.tile_pool(name="xin", bufs=1) as xin,
            tc.tile_pool(name="proj", bufs=1) as proj,
            tc.tile_pool(name="epool", bufs=E_BUFS) as epool,
            tc.tile_pool(name="vfpool", bufs=VF_BUFS) as vfpool,
            tc.tile_pool(name="scrpool", bufs=2) as scrpool,
            tc.tile_pool(name="vspool", bufs=VS_BUFS) as vspool,
            tc.tile_pool(name="stats", bufs=1) as stats,
            tc.tile_pool(name="outp", bufs=1) as outp,
            tc.tile_pool(name="spsum", bufs=2, space="PSUM") as spsum,
            tc.tile_pool(name="avpsum", bufs=1, space="PSUM") as avpsum,
        ):
            wq_s = consts.tile([P, NDC, P], bf16)
            wk_s = consts.tile([P, NDC, P], bf16)
            wv_s = consts.tile([P, NDC, P], bf16)
            qt_s = xin.tile([P, NDC, L], bf16)
            kt_s = xin.tile([P, NDC, L], bf16)
            vt_s = xin.tile([P, NDC, L], bf16)
            qt_r = qt_d.rearrange("(o p) l -> p o l", p=P)
            kt_r = kt_d.rearrange("(o p) l -> p o l", p=P)
            vt_r = vt_d.rearrange("(o p) l -> p o l", p=P)

            def load_chunk(sb, rr, c):
                nc.sync.dma_start(sb[:, :, ts(c, 512)], rr[:, :, ts(c, 512)])

            # PE p-state: tiny dummy matmul at t~0 starts the 3us ramp early
            zt = consts.tile([P, P], bf16)
            nc.gpsimd.memset(zt[:], 0.0)
            pwp = spsum.tile([P, 1024], f32, tag="sco", name="prewarm")
            nc.tensor.matmul(pwp[:, 0:P], lhsT=zt[:], rhs=zt[:], start=True, stop=True)

            # critical-path-first load order
            nc.sync.dma_start(wq_s[:], wq_d.rearrange("(o p) e -> p o e", p=P))
            load_chunk(qt_s, qt_r, 0)
            nc.sync.dma_start(wk_s[:], wk_d.rearrange("(o p) e -> p o e", p=P))
            load_chunk(kt_s, kt_r, 0)
            load_chunk(qt_s, qt_r, 1)
            load_chunk(qt_s, qt_r, 2)
            load_chunk(qt_s, qt_r, 3)
            nc.sync.dma_start(wv_s[:], wv_d.rearrange("(o p) e -> p o e", p=P))
            load_chunk(vt_s, vt_r, 0)
            load_chunk(kt_s, kt_r, 1)
            load_chunk(kt_s, kt_r, 2)
            load_chunk(vt_s, vt_r, 1)
            load_chunk(kt_s, kt_r, 3)
            load_chunk(vt_s, vt_r, 2)
            load_chunk(vt_s, vt_r, 3)

            # fp8 projected Q/K: head h features on partitions h*64..h*64+64
            QT8 = proj.tile([P, L], fp8)
            KT8 = proj.tile([P, L], fp8)

            # per-(kt,h) denominator sums; [P, kt*2 + h] pairs adjacent for
            # the reciprocal
            Dsum2 = stats.tile([P, NKT, 2, 2], f32)  # ACT accum halves [kt,h,hf]
            Dtot = stats.tile([P, NKT * 2], f32)
            Drec = stats.tile([P, NKT * 2], f32)

            # AV accumulator: out[q(128), qt(16), ev-pack(128)] f32 = 4 banks
            avpA = avpsum.tile([P, 8, P], f32, tag="ava")
            avpB = avpsum.tile([P, 8, P], f32, tag="avb")

            def qk_proj_half(W, X, OUT8, hf, on_act=True):
                # project q-cols [hf*1024, hf*1024+1024): 2 chunks of 512
                ps = spsum.tile([P, 1024], f32, tag="sco", name="qkproj")
                for sub in range(2):
                    qc = hf * 2 + sub
                    for dc in range(NDC):
                        nc.tensor.matmul(
                            ps[:, ts(sub, 512)],
                            lhsT=W[:, dc, :],
                            rhs=X[:, dc, ts(qc, 512)],
                            start=(dc == 0),
                            stop=(dc == NDC - 1),
                        )
                if on_act:
                    nc.scalar.copy(OUT8[:, ts(hf, 1024)], ps[:])
                else:
                    nc.vector.tensor_copy(OUT8[:, ts(hf, 1024)], ps[:])

            Etiles = {}
            Vstiles = {}

            def scores_half(kt, h, hf, E):
                # one [128, 1024] half: 2 DoubleRow fp8 matmuls + one exp
                hp = h * DH
                ps = spsum.tile([P, 1024], f32, tag="sco", name="sco")
                for sub in range(2):
                    qc = hf * 2 + sub
                    lhs = (
                        KT8[hp : hp + DH, ts(kt, P)]
                        .unsqueeze(1)
                        .broadcast_to([DH, 2, P])
                    )
                    rhs = (
                        QT8[hp : hp + DH, ts(qc, 512)]
                        .unsqueeze(1)
                        .broadcast_to([DH, 2, 512])
                    )
                    nc.tensor.matmul(
                        ps[:, ts(sub, 512)],
                        lhsT=lhs,
                        rhs=rhs,
                        start=True,
                        stop=True,
                        perf_mode=DR,
                    )
                if (kt, h) in ACT_BLOCKS:
                    nc.scalar.activation(
                        E[:, ts(hf, 1024)],
                        ps[:],
                        AF.Exp,
                        scale=SCALE2,
                        accum_out=Dsum2[:, kt : kt + 1, h : h + 1, hf : hf + 1],
                    )
                else:
                    e16 = E[:, ts(hf, 1024)].bitcast(i16)
                    nc.vector.tensor_scalar(
                        e16, ps[:], EXP_A2, EXP_B, ALU.mult, ALU.add
                    )

            def evac_half_a():
                oa = outp.tile([P, 8, P], bf16, tag="oca")
                nc.scalar.copy(oa[:], avpA[:])
                nc.scalar.dma_start(out_d[:, 0:8, :], oa[:])

            def evac_half_b():
                ob = outp.tile([P, 8, P], bf16, tag="ocb")
                nc.vector.tensor_copy(ob[:], avpB[:])
                nc.sync.dma_start(out_d[:, 8:16, :], ob[:])

            def av_chunk(kt, qlo, qhi, tail=False):
                E0, E1 = Etiles[kt]
                Vs = Vstiles[kt]
                for qt in range(qlo, qhi):
                    av = avpA if qt < 8 else avpB
                    qi = qt % 8
                    for h in range(2):
                        E = E0 if h == 0 else E1
                        # start=True zeroes the whole PSUM bank (4 q-tiles),
                        # so only the first matmul per bank may carry it
                        nc.tensor.matmul(
                            av[:, qi, ts(h, DH)],
                            lhsT=E[:, ts(qt, P)],
                            rhs=Vs[:, ts(h, DH)],
                            start=(kt == 0 and h == 0 and qt % 4 == 0),
                            stop=(kt == NKT - 1),
                            skip_group_check=True,
                        )
                    if tail and qt == 7:
                        evac_half_a()

            def vproj_group(g):
                # V projection for k-tiles 4g..4g+3 in one [128,512] psum
                psv = spsum.tile([P, 1024], f32, tag="sco", name="psv")
                for j in range(4):
                    # j=0 / j=2 start=True zeroes banks 0 / 1 of the tile;
                    # j=1 / j=3 accumulate onto the already-zeroed bank
                    for dc in range(NDC):
                        nc.tensor.matmul(
                            psv[:, ts(j, P)],
                            lhsT=vt_s[:, dc, ts(4 * g + j, P)],
                            rhs=wv_s[:, dc, :],
                            start=(j % 2 == 0 and dc == 0),
                            stop=(dc == NDC - 1),
                            skip_group_check=True,
                        )
                Vf = vfpool.tile([P, 512], bf16, tag="vf")
                nc.scalar.copy(Vf[:], psv[:, 0:512])
                return Vf

            def d_vs_block(kt, Vf, voff):
                # D: ACT blocks from fused accum halves (Pool adds them);
                # DVE blocks from a 4x bf16 accum pass over E.  Reciprocals
                # are split per head with the DVE-dependency-only head first,
                # so DVE's in-order queue never fences on the (saturated)
                # ACT -> Pool-add chain.
                order = []
                for h in range(2):
                    E = Etiles[kt][h]
                    if (kt, h) in ACT_BLOCKS:
                        nc.gpsimd.tensor_add(
                            Dtot[:, 2 * kt + h : 2 * kt + h + 1],
                            Dsum2[:, kt, h, 0:1],
                            Dsum2[:, kt, h, 1:2],
                        )
                        order.append(h)
                    else:
                        # separate scr output (not in-place): keeps AV's E
                        # reads off the D-pass dependency
                        scr = scrpool.tile([P, L], bf16, tag="scr")
                        nc.vector.tensor_scalar(
                            scr[:],
                            E[:],
                            1.0,
                            0.0,
                            ALU.mult,
                            ALU.add,
                            accum_out=Dtot[:, 2 * kt + h : 2 * kt + h + 1],
                        )
                        order.insert(0, h)
                for h in order:
                    nc.vector.reciprocal(
                        Drec[:, 2 * kt + h : 2 * kt + h + 1],
                        Dtot[:, 2 * kt + h : 2 * kt + h + 1],
                    )
                Vs = vspool.tile([P, P], bf16, tag="vs")
                Vstiles[kt] = Vs
                for h in order:
                    # Pool: Vs = Vf * (1/D), per-partition ptr scalar
                    nc.gpsimd.tensor_scalar_mul(
                        Vs[:, ts(h, DH)],
                        Vf[:, voff + h * DH : voff + (h + 1) * DH],
                        Drec[:, 2 * kt + h : 2 * kt + h + 1],
                    )

            def alloc_E(kt):
                E0 = epool.tile([P, L], bf16, tag="E", name=f"E{kt}h0")
                E1 = epool.tile([P, L], bf16, tag="E", name=f"E{kt}h1")
                Etiles[kt] = (E0, E1)

            # warmup: Q halves then K halves rotate through the 2 spsum bufs
            qk_proj_half(wq_s, qt_s, QT8, 0, on_act=True)
            qk_proj_half(wq_s, qt_s, QT8, 1, on_act=True)
            qk_proj_half(wk_s, kt_s, KT8, 0, on_act=True)

            Vfgroups = {}

            for kt in range(NKT):
                alloc_E(kt)
                E0, E1 = Etiles[kt]
                if kt == 0:
                    scores_half(0, 0, 0, E0)
                    qk_proj_half(wk_s, kt_s, KT8, 1, on_act=False)
                    scores_half(0, 0, 1, E0)
                    scores_half(0, 1, 0, E1)
                    scores_half(0, 1, 1, E1)
                    Vfgroups[0] = vproj_group(0)
                    d_vs_block(0, Vfgroups[0], 0)
                else:
                    # interleave: AV chunks of kt-1 fill the PE queue while
                    # the exp engines drain the first-half psum tiles, so the
                    # in-order PE never parks ahead of ready AV work
                    scores_half(kt, 0, 0, E0)
                    scores_half(kt, 1, 0, E1)
                    av_chunk(kt - 1, 0, 8)
                    scores_half(kt, 0, 1, E0)
                    scores_half(kt, 1, 1, E1)
                    g, r = divmod(kt + 1, 4)
                    if r == 0 and g < 4:
                        Vfgroups[g] = vproj_group(g)
                    av_chunk(kt - 1, 8, 16)
                    d_vs_block(kt, Vfgroups[kt // 4], (kt % 4) * P)

            av_chunk(NKT - 1, 0, 8, tail=True)
            av_chunk(NKT - 1, 8, 16, tail=True)
            evac_half_b()

    nc.compile()
    return nc


def _get_program():
    if "nc" not in _CACHE:
        _CACHE["nc"] = _build_program()
    return _CACHE["nc"]


def kernel(keys, queries, values, WQ, WK, WV):
    import ml_dtypes

    from concourse import bass_utils

    bf = ml_dtypes.bfloat16
    keys = np.asarray(keys)
    queries = np.asarray(queries)
    values = np.asarray(values)
    WQ = np.asarray(WQ)
    WK = np.asarray(WK)
    WV = np.asarray(WV)

    nc = _get_program()

    in_maps = []
    for c in range(N_CORES):
        b = c // 4
        h0 = 2 * (c % 4)
        h1 = h0 + 1
        in_maps.append(
            {
                "qt": np.ascontiguousarray(queries[b].T).astype(bf),
                "kt": np.ascontiguousarray(keys[b].T).astype(bf),
                "vt": np.ascontiguousarray(values[b].T).astype(bf),
                "wq": np.concatenate([WQ[h0], WQ[h1]], axis=1).astype(bf),
                "wk": np.concatenate([WK[h0], WK[h1]], axis=1).astype(bf),
                "wv": np.concatenate([WV[h0], WV[h1]], axis=1).astype(bf),
            }
        )

    res = bass_utils.run_bass_kernel_spmd(nc, in_maps, core_ids=list(range(N_CORES)))

    out = np.empty((B, L, H * DH), dtype=np.float32)
    for c in range(N_CORES):
        b = c // 4
        h0 = 2 * (c % 4)
        ot = np.asarray(res.results[c]["out"], dtype=np.float32)  # [128,16,128]
        out[b, :, h0 * DH : (h0 + 2) * DH] = ot.transpose(1, 0, 2).reshape(L, P)
    return out


# revision 18
# speedup vs baseline: 1.0990x; 1.0632x over previous
"""MultiHeadAttention Trainium2 kernel (8 NeuronCores, SPMD), v2.

Problem: B=2, L=2048, DK=DV=512, H=8, dh=64.
  Q = q @ WQ[h]; K = k @ WK[h]; V = v @ WV[h]       (per head)
  y = Q K^T / sqrt(L); z = softmax(y, axis=QUERY); out = z @ V
  concat heads on feature dim.

Sharding: 16 (b,h) pairs over 8 cores -> 2 heads (same batch) per core.

v2 changes vs v1 (81.3us):
  * Scores matmuls run in fp8e4m3 with perf_mode=DoubleRow at 0.5
    cycles/row (half the bf16 PE time).  The 64-deep head contraction is
    packed to the required 2-per-partition pair layout with stride-0
    broadcast APs (each feature streamed twice -> computes 2*y; the 1/2
    is folded into the exp scale).  Q/K projections evacuate straight to
    fp8 (no bf16 Q/K ever materialized).
  * Scores PSUM tiles are [128, 1024] (2 banks); exp runs as halves,
    amortizing the fixed ACT/DVE access-latency + accum-read costs.
  * exp blocks (kt, h) are split ACT-exact vs DVE-Schraudolph via
    ACT_BLOCKS; DVE block D-sums run in-place (out == in) at 4x mode.
  * V projection batched 4 k-tiles per PSUM tile, one [128,512] ACT
    evac instead of 4 x [128,128].
  * Vs = Vf * (1/D) scaling moved to the (otherwise idle) Pool engine.
  * PSUM: 2 x [128,1024] score tiles (also hosting Q/K/V projections)
    + 4-bank AV accumulator = exactly 8 banks.
"""

import math

import numpy as np

B = 2
L = 2048
DK = 512
H = 8
DH = 64
P = 128
NKT = L // P  # 16 k-tiles
NDC = DK // P  # 4 d-chunks
N_CORES = 8

SCALE = 1.0 / math.sqrt(float(L))
# Schraudolph in bf16-bits domain: round(raw*EXP_A2 + EXP_B) as int16,
# bitcast bf16 ~= exp(raw*SCALE).  Raw fp8 scores are 2*y so EXP_A2
# carries a 1/2.
EXP_A2 = 128.0 * math.log2(math.e) * SCALE * 0.5
EXP_B = 16256.0 - 12.0
SCALE2 = SCALE * 0.5  # ACT exp scale on the doubled raw scores

# (kt, h) blocks whose exp runs exact on ACT with fused accum D; the
# rest run DVE Schraudolph (~2% rel, bias common-mode cancels in E/D).
ACT_BLOCKS = frozenset((kt, 0) for kt in range(NKT))

E_BUFS = 6
VF_BUFS = 2
VS_BUFS = 4

_CACHE = {}


def _build_program():
    import concourse.bass as bass
    import concourse.tile as tile
    from concourse import bacc, mybir
    from concourse.bass import ts

    f32 = mybir.dt.float32
    bf16 = mybir.dt.bfloat16
    fp8 = mybir.dt.float8e4
    i16 = mybir.dt.int16
    AF = mybir.ActivationFunctionType
    ALU = mybir.AluOpType
    DR = mybir.MatmulPerfMode.DoubleRow

    nc = bacc.Bacc("TRN2", target_bir_lowering=False, debug=False)

    qt_d = nc.dram_tensor("qt", [DK, L], bf16, kind="ExternalInput")
    kt_d = nc.dram_tensor("kt", [DK, L], bf16, kind="ExternalInput")
    vt_d = nc.dram_tensor("vt", [DK, L], bf16, kind="ExternalInput")
    wq_d = nc.dram_tensor("wq", [DK, P], bf16, kind="ExternalInput")
    wk_d = nc.dram_tensor("wk", [DK, P], bf16, kind="ExternalInput")
    wv_d = nc.dram_tensor("wv", [DK, P], bf16, kind="ExternalInput")
    out_d = nc.dram_tensor("out", [P, NKT, P], bf16, kind="ExternalOutput")

    with tile.TileContext(nc) as tc:
        with (
            tc.tile_pool(name="consts", bufs=1) as consts,
            tc.tile_pool(name="xin", bufs=1) as xin,
            tc.tile_pool(name="proj", bufs=1) as proj,
            tc.tile_pool(name="epool", bufs=E_BUFS) as epool,
            tc.tile_pool(name="vfpool", bufs=VF_BUFS) as vfpool,
            tc.tile_pool(name="scrpool", bufs=2) as scrpool,
            tc.tile_pool(name="vspool", bufs=VS_BUFS) as vspool,
            tc.tile_pool(name="stats", bufs=1) as stats,
            tc.tile_pool(name="outp", bufs=1) as outp,
            tc.tile_pool(name="spsum", bufs=2, space="PSUM") as spsum,
            tc.tile_pool(name="avpsum", bufs=1, space="PSUM") as avpsum,
        ):
            wq_s = consts.tile([P, NDC, P], bf16)
            wk_s = consts.tile([P, NDC, P], bf16)
            wv_s = consts.tile([P, NDC, P], bf16)
            qt_s = xin.tile([P, NDC, L], bf16)
            kt_s = xin.tile([P, NDC, L], bf16)
            vt_s = xin.tile([P, NDC, L], bf16)
            qt_r = qt_d.rearrange("(o p) l -> p o l", p=P)
            kt_r = kt_d.rearrange("(o p) l -> p o l", p=P)
            vt_r = vt_d.rearrange("(o p) l -> p o l", p=P)

            def load_chunk(sb, rr, c):
                nc.sync.dma_start(sb[:, :, ts(c, 512)], rr[:, :, ts(c, 512)])

            # PE p-state: tiny dummy matmul at t~0 starts the 3us ramp early
            zt = consts.tile([P, P], bf16)
            nc.gpsimd.memset(zt[:], 0.0)
            pwp = spsum.tile([P, 1024], f32, tag="sco", name="prewarm")
            nc.tensor.matmul(pwp[:, 0:P], lhsT=zt[:], rhs=zt[:], start=True, stop=True)

            # critical-path-first load order
            nc.sync.dma_start(wq_s[:], wq_d.rearrange("(o p) e -> p o e", p=P))
            load_chunk(qt_s, qt_r, 0)
            nc.sync.dma_start(wk_s[:], wk_d.rearrange("(o p) e -> p o e", p=P))
            load_chunk(kt_s, kt_r, 0)
            load_chunk(qt_s, qt_r, 1)
            load_chunk(qt_s, qt_r, 2)
            load_chunk(qt_s, qt_r, 3)
            nc.sync.dma_start(wv_s[:], wv_d.rearrange("(o p) e -> p o e", p=P))
            load_chunk(vt_s, vt_r, 0)
            load_chunk(kt_s, kt_r, 1)
            load_chunk(kt_s, kt_r, 2)
            load_chunk(vt_s, vt_r, 1)
            load_chunk(kt_s, kt_r, 3)
            load_chunk(vt_s, vt_r, 2)
            load_chunk(vt_s, vt_r, 3)

            # fp8 projected Q/K: head h features on partitions h*64..h*64+64
            QT8 = proj.tile([P, L], fp8)
            KT8 = proj.tile([P, L], fp8)

            # per-(kt,h) denominator sums; [P, kt*2 + h] pairs adjacent for
            # the reciprocal
            Dsum2 = stats.tile([P, NKT, 2, 2], f32)  # ACT accum halves [kt,h,hf]
            Dtot = stats.tile([P, NKT * 2], f32)
            Drec = stats.tile([P, NKT * 2], f32)

            # AV accumulator: out[q(128), qt(16), ev-pack(128)] f32 = 4 banks
            avpA = avpsum.tile([P, 8, P], f32, tag="ava")
            avpB = avpsum.tile([P, 8, P], f32, tag="avb")

            def qk_proj_quarter(W, X, OUT8, qc, on_act=True):
                # project cols [qc*512, qc*512+512) -- needs only input
                # chunk qc, so warmup evacs start as soon as DMAs land
                ps = spsum.tile([P, 1024], f32, tag="sco", name="qkproj")
                for dc in range(NDC):
                    nc.tensor.matmul(
                        ps[:, 0:512],
                        lhsT=W[:, dc, :],
                        rhs=X[:, dc, ts(qc, 512)],
                        start=(dc == 0),
                        stop=(dc == NDC - 1),
                    )
                if on_act:
                    nc.scalar.copy(OUT8[:, ts(qc, 512)], ps[:, 0:512])
                else:
                    nc.vector.tensor_copy(OUT8[:, ts(qc, 512)], ps[:, 0:512])

            Etiles = {}
            Vstiles = {}

            def scores_half(kt, h, hf, E):
                # one [128, 1024] half: 2 DoubleRow fp8 matmuls + one exp
                hp = h * DH
                ps = spsum.tile([P, 1024], f32, tag="sco", name="sco")
                for sub in range(2):
                    qc = hf * 2 + sub
                    lhs = (
                        KT8[hp : hp + DH, ts(kt, P)]
                        .unsqueeze(1)
                        .broadcast_to([DH, 2, P])
                    )
                    rhs = (
                        QT8[hp : hp + DH, ts(qc, 512)]
                        .unsqueeze(1)
                        .broadcast_to([DH, 2, 512])
                    )
                    nc.tensor.matmul(
                        ps[:, ts(sub, 512)],
                        lhsT=lhs,
                        rhs=rhs,
                        start=True,
                        stop=True,
                        perf_mode=DR,
                    )
                if (kt, h) in ACT_BLOCKS:
                    nc.scalar.activation(
                        E[:, ts(hf, 1024)],
                        ps[:],
                        AF.Exp,
                        scale=SCALE2,
                        accum_out=Dsum2[:, kt : kt + 1, h : h + 1, hf : hf + 1],
                    )
                else:
                    e16 = E[:, ts(hf, 1024)].bitcast(i16)
                    nc.vector.tensor_scalar(
                        e16, ps[:], EXP_A2, EXP_B, ALU.mult, ALU.add
                    )

            def evac_half_a():
                oa = outp.tile([P, 8, P], bf16, tag="oca")
                nc.scalar.copy(oa[:], avpA[:])
                nc.scalar.dma_start(out_d[:, 0:8, :], oa[:])

            def evac_half_b():
                ob = outp.tile([P, 8, P], bf16, tag="ocb")
                nc.vector.tensor_copy(ob[:], avpB[:])
                nc.sync.dma_start(out_d[:, 8:16, :], ob[:])

            def av_part(kt, h, qlo, qhi):
                # AV matmuls for ONE head: h1 is emitted early (its Vs closes
                # during kt), h0 late (its Vs closes early in kt+1), so PE
                # never parks on a not-yet-scaled Vs
                E = Etiles[kt][h]
                Vs = Vstiles[kt]
                for qt in range(qlo, qhi):
                    av = avpA if qt < 8 else avpB
                    qi = qt % 8
                    # start=True zeroes the whole PSUM bank (4 q-tiles): only
                    # the first-emitted head (h1) on the first kt carries it
                    nc.tensor.matmul(
                        av[:, qi, ts(h, DH)],
                        lhsT=E[:, ts(qt, P)],
                        rhs=Vs[:, ts(h, DH)],
                        start=(kt == 0 and h == 1 and qt % 4 == 0),
                        stop=(kt == NKT - 1),
                        skip_group_check=True,
                    )

            def vproj_group(g):
                # V projection for k-tiles 4g..4g+3 in one [128,512] psum
                psv = spsum.tile([P, 1024], f32, tag="sco", name="psv")
                for j in range(4):
                    # j=0 / j=2 start=True zeroes banks 0 / 1 of the tile;
                    # j=1 / j=3 accumulate onto the already-zeroed bank
                    for dc in range(NDC):
                        nc.tensor.matmul(
                            psv[:, ts(j, P)],
                            lhsT=vt_s[:, dc, ts(4 * g + j, P)],
                            rhs=wv_s[:, dc, :],
                            start=(j % 2 == 0 and dc == 0),
                            stop=(dc == NDC - 1),
                            skip_group_check=True,
                        )
                Vf = vfpool.tile([P, 512], bf16, tag="vf")
                nc.scalar.copy(Vf[:], psv[:, 0:512])
                return Vf

            Vfinfo = {}

            def stats_early(kt):
                # h0 (ACT block) chain wrap-up for kt, run one iteration
                # LATE so every dependency (ACT accum aux -> Pool add) is
                # already stale when DVE's in-order queue reaches the
                # reciprocal: no cross-engine fence.
                if kt < 0:
                    return
                Vf, voff = Vfinfo[kt]
                nc.vector.reciprocal(
                    Drec[:, 2 * kt : 2 * kt + 1], Dtot[:, 2 * kt : 2 * kt + 1]
                )
                nc.gpsimd.tensor_scalar_mul(
                    Vstiles[kt][:, 0:DH],
                    Vf[:, voff : voff + DH],
                    Drec[:, 2 * kt : 2 * kt + 1],
                )

            def stats_late(kt):
                # h1 (DVE block) chain: all-DVE deps, closes within kt.
                # The Pool add for h0's accum halves is emitted LAST so it
                # cannot delay Vs_h1 on Pool's in-order queue.
                Vf, voff = Vfinfo[kt]
                scr = scrpool.tile([P, L], bf16, tag="scr")
                nc.vector.tensor_scalar(
                    scr[:],
                    Etiles[kt][1][:],
                    1.0,
                    0.0,
                    ALU.mult,
                    ALU.add,
                    accum_out=Dtot[:, 2 * kt + 1 : 2 * kt + 2],
                )
                nc.vector.reciprocal(
                    Drec[:, 2 * kt + 1 : 2 * kt + 2],
                    Dtot[:, 2 * kt + 1 : 2 * kt + 2],
                )
                Vs = vspool.tile([P, P], bf16, tag="vs")
                Vstiles[kt] = Vs
                nc.gpsimd.tensor_scalar_mul(
                    Vs[:, DH : 2 * DH],
                    Vf[:, voff + DH : voff + 2 * DH],
                    Drec[:, 2 * kt + 1 : 2 * kt + 2],
                )
                nc.gpsimd.tensor_add(
                    Dtot[:, 2 * kt : 2 * kt + 1],
                    Dsum2[:, kt, 0, 0:1],
                    Dsum2[:, kt, 0, 1:2],
                )

            def alloc_E(kt):
                E0 = epool.tile([P, L], bf16, tag="E", name=f"E{kt}h0")
                E1 = epool.tile([P, L], bf16, tag="E", name=f"E{kt}h1")
                Etiles[kt] = (E0, E1)

            # warmup: 512-wide projection quarters gated only on their own
            # input chunk, so ACT starts evacuating ~2us in
            qk_proj_quarter(wq_s, qt_s, QT8, 0, on_act=True)
            qk_proj_quarter(wk_s, kt_s, KT8, 0, on_act=True)
            qk_proj_quarter(wq_s, qt_s, QT8, 1, on_act=True)

            for kt in range(NKT):
                alloc_E(kt)
                E0, E1 = Etiles[kt]
                if kt == 0:
                    scores_half(0, 0, 0, E0)
                    scores_half(0, 1, 0, E1)
                    qk_proj_quarter(wq_s, qt_s, QT8, 2, on_act=True)
                    qk_proj_quarter(wq_s, qt_s, QT8, 3, on_act=True)
                    scores_half(0, 0, 1, E0)
                    scores_half(0, 1, 1, E1)
                    Vf0 = vproj_group(0)
                    for j in range(4):
                        Vfinfo[j] = (Vf0, j * P)
                    stats_late(0)
                else:
                    scores_half(kt, 0, 0, E0)
                    scores_half(kt, 1, 0, E1)
                    stats_early(kt - 1)
                    if kt - 1 >= 0:
                        av_part(kt - 1, 1, 0, 16)
                    scores_half(kt, 0, 1, E0)
                    scores_half(kt, 1, 1, E1)
                    if kt <= 3:
                        qk_proj_quarter(wk_s, kt_s, KT8, kt, on_act=True)
                    g, r = divmod(kt + 1, 4)
                    if r == 0 and g < 4:
                        Vf = vproj_group(g)
                        for j in range(4):
                            Vfinfo[4 * g + j] = (Vf, j * P)
                    av_part(kt - 1, 0, 0, 16)
                    stats_late(kt)

            # tail: close kt=15's chains, then AV with the first-half evac
            # overlapping the second half's matmuls
            stats_early(NKT - 1)
            av_part(NKT - 1, 1, 0, 8)
            av_part(NKT - 1, 0, 0, 8)
            evac_half_a()
            av_part(NKT - 1, 1, 8, 16)
            av_part(NKT - 1, 0, 8, 16)
            evac_half_b()

    nc.compile()
    return nc


def _get_program():
    if "nc" not in _CACHE:
        _CACHE["nc"] = _build_program()
    return _CACHE["nc"]


def kernel(keys, queries, values, WQ, WK, WV):
    import ml_dtypes

    from concourse import bass_utils

    bf = ml_dtypes.bfloat16
    keys = np.asarray(keys)
    queries = np.asarray(queries)
    values = np.asarray(values)
    WQ = np.asarray(WQ)
    WK = np.asarray(WK)
    WV = np.asarray(WV)

    nc = _get_program()

    in_maps = []
    for c in range(N_CORES):
        b = c // 4
        h0 = 2 * (c % 4)
        h1 = h0 + 1
        in_maps.append(
            {
                "qt": np.ascontiguousarray(queries[b].T).astype(bf),
                "kt": np.ascontiguousarray(keys[b].T).astype(bf),
                "vt": np.ascontiguousarray(values[b].T).astype(bf),
                "wq": np.concatenate([WQ[h0], WQ[h1]], axis=1).astype(bf),
                "wk": np.concatenate([WK[h0], WK[h1]], axis=1).astype(bf),
                "wv": np.concatenate([WV[h0], WV[h1]], axis=1).astype(bf),
            }
        )

    res = bass_utils.run_bass_kernel_spmd(nc, in_maps, core_ids=list(range(N_CORES)))

    out = np.empty((B, L, H * DH), dtype=np.float32)
    for c in range(N_CORES):
        b = c // 4
        h0 = 2 * (c % 4)
        ot = np.asarray(res.results[c]["out"], dtype=np.float32)  # [128,16,128]
        out[b, :, h0 * DH : (h0 + 2) * DH] = ot.transpose(1, 0, 2).reshape(L, P)
    return out
